# revision 9
# baseline (speedup 1.0000x reference)
"""Causal self-attention (softmax over the QUERY axis) for Trainium2, 8 cores.

Reference semantics (note the quirk -- softmax over dim=1, the query axis):
    q = x @ Wq.T ; k = x @ Wk.T ; v = x @ Wv.T          (per batch)
    s[q_, k_] = <q[q_], k[k_]>,  masked -inf where k_ > q_
    attn = softmax(s / sqrt(D), axis=q_)                 (normalize per key column)
    out[q_, :] = sum_k attn[q_, k_] * v[k_, :]

Because the softmax normalizes each key COLUMN over queries, the whole thing
factors as  out = W @ (v / Z)  with
    W[k_, q_] = exp(s^T * scale) * causal_mask,   Z[k_] = sum_q W[k_, q_].

Algebraic folding: s[q_, k_] = x[q_] . A . x[k_]  with A = Wq^T @ Wk, so with
y = x_k @ A^T the scores come straight from x (no q/k projections needed):
    s^T[k_, q_] = sum_d y[k_, d] * x[q_, d].
A is computed once on the host.

Sharding: 8 cores = 4 batches x 2 key-shards.  Key columns are interleaved by
parity (core h in {0,1} owns original key positions 2*m + h) so the causal
work balances AND every core runs the identical program (pure SPMD); only the
input data differs per core.  Each core computes a partial output (sum over
its own keys); the host adds the two partials per batch.

Device layout (per core, b = batch, h = parity):
    xT   [D, N]  bf16  x[b].T
    a2   [D, D]  bf16  A^T = Wk^T @ Wq   (layout [e, d])
    wvT  [D, D]  bf16  Wv.T              (layout [e, o])
    maskbias [128, 256] f32  0 where valid, -1e9 where masked (depends on h)
    out  [N, D]  f32   partial output

Pipeline structure (v2):
  - input DMAs interleaved (xk[e]/a2[e] pairs first) so the y-projection's
    accumulation chains start as soon as the first pair lands;
  - v-projection moved inside the per-group loop, accumulated in a 2-bank
    PSUM tile and scaled by 1/Z directly PSUM->SBUF(bf16) on the Scalar
    engine (activation Copy with per-partition scale) -- no fp32 v copy;
  - AV for q-tiles of group g-1 emitted at group g (software pipelining);
  - LDWEIGHTS sharing: consecutive matmuls reuse the same stationary
    operand where possible (y-proj / v-proj / AV emit both free-dim chunks
    per stationary block).

All matmul inputs are bf16 (PE full rate), accumulation fp32 in PSUM.
"""
import numpy as np
import ml_dtypes
from contextlib import ExitStack

import concourse.bass as bass
import concourse.tile as tile
import concourse.bacc as bacc
import concourse.mybir as mybir
from concourse.bass_utils import run_bass_kernel_spmd

B, N, D = 4, 2048, 1024
NT = N // 128          # 16 query tiles
ET = D // 128          # 8 contraction tiles
G = 8                  # key groups per core (128 interleaved keys each)
SCALE = 1.0 / np.sqrt(D).astype(np.float32)
NEGBIG = -1.0e9

BF = mybir.dt.bfloat16
F32 = mybir.dt.float32

# packed offsets of each group's score row-block inside the wT buffer
WOFF = []
_o = 0
for _g in range(G):
    WOFF.append(_o)
    _o += N - 256 * _g
WTOT = _o  # 9216


def _score_chunks(g):
    """(q0, width) chunks covering the valid span [256g, N) of group g.
    The first chunk always contains the 256 masked columns; widths 512/256."""
    width = N - 256 * g
    q0 = 256 * g
    chunks = []
    if (width // 256) % 2 == 1:
        chunks.append((q0, 256))
        q0 += 256
    while q0 < N:
        chunks.append((q0, 512))
        q0 += 512
    return chunks


def _emit_body(nc, tc, ctx, pools, aps, stages="full"):
    (xkpool, xtpool, wpool, ypool, vppool, zpool, stpool, ps, vps) = pools
    (xT_d, xkT_d, a2_d, wvT_d, mb_sb, wt_sb, out_d) = aps
    full = stages == "full"
    Exp = mybir.ActivationFunctionType.Exp
    Copy = mybir.ActivationFunctionType.Copy

    # ---- PE warm-up: dummy matmuls on a zeroed tile keep the PE HAM busy
    #      while the first input DMAs land, so real matmuls run at 2.4GHz ----
    warm = zpool.tile([128, 512], BF, tag="warm", name="warm")
    nc.gpsimd.memset(warm[:], 0)
    wps = ps.tile([128, 512], F32, tag="ps", name="wps")
    NWARM = 8
    for i in range(NWARM):
        nc.tensor.matmul(wps[:], warm[:, :128], warm[:],
                         start=(i == 0), stop=(i == NWARM - 1))

    # ---- loads: xk/a2 pairs (y-proj can start after the first pair),
    #      then xT (needed by scores at ~27us), then wvT (v-proj) ----
    xk, a2t = [], []
    for e in range(ET):
        t1 = xkpool.tile([128, D], BF, tag="xk")
        nc.sync.dma_start(t1[:], xkT_d[e * 128:(e + 1) * 128, :])
        xk.append(t1)
        t2 = wpool.tile([128, D], BF, tag="a2")
        nc.sync.dma_start(t2[:], a2_d[e * 128:(e + 1) * 128, :])
        a2t.append(t2)
    xt = []
    for t in range(ET):
        xtile = xtpool.tile([128, N], BF, tag="xt")
        nc.sync.dma_start(xtile[:], xT_d[t * 128:(t + 1) * 128, :])
        xt.append(xtile)
    wvt = []
    for e in range(ET):
        w = wpool.tile([128, D], BF, tag="wv")
        nc.sync.dma_start(w[:], wvT_d[e * 128:(e + 1) * 128, :])
        wvt.append(w)

    # ---- y projection: yT[dt_][:, m] = sum_e a2[e, d_blk] * xkT[e, m] ----
    # dt-major; both 512-wide m-chunks per stationary block (LDW shared).
    yt = []
    for dt_ in range(ET):
        ytile = ypool.tile([128, D], BF, tag="yt")
        pts = [ps.tile([128, 512], F32, tag="ps", name=f"pt{c}") for c in range(2)]
        for e in range(ET):
            for c in range(2):
                nc.tensor.matmul(
                    pts[c][:],
                    a2t[e][:, dt_ * 128:(dt_ + 1) * 128],
                    xk[e][:, c * 512:(c + 1) * 512],
                    start=(e == 0), stop=(e == ET - 1),
                )
        if full:
            for c in range(2):
                nc.vector.tensor_copy(ytile[:, c * 512:(c + 1) * 512], pts[c][:])
        yt.append(ytile)

    vp = [None] * G
    stage_tiles = {}

    def emit_av(qt):
        """AV for q-tile qt: out[128q, D] = sum_gg wt_gg^T @ vp_gg."""
        g_qt = qt // 2
        stage = stpool.tile([128, D], F32, tag="st")
        apts = [ps.tile([128, 512], F32, tag="ps", name=f"pt{c}") for c in range(2)]
        # For the very last q-tile, run the two oc-chains back-to-back
        # (oc-outer) so oc0's copy+DMA overlaps oc1's matmuls; elsewhere
        # interleave them (gg-outer) to share the stationary operand.
        if qt == NT - 1:
            order = [(gg, oc) for oc in range(2) for gg in range(g_qt + 1)]
        else:
            order = [(gg, oc) for gg in range(g_qt + 1) for oc in range(2)]
        for gg, oc in order:
            if full:
                lhs = wt_sb[:, WOFF[gg] + 128 * qt - 256 * gg:
                               WOFF[gg] + 128 * qt - 256 * gg + 128]
                rhs = vp[gg][:, oc * 512:(oc + 1) * 512]
            else:
                lhs = xk[gg][:, :128]
                rhs = xk[gg][:, oc * 512:(oc + 1) * 512]
            nc.tensor.matmul(apts[oc][:], lhs, rhs,
                             start=(gg == 0), stop=(gg == g_qt))
        if full or qt == NT - 1:
            # oc0 staged on DVE, oc1 on ACT (parallel), separate out-DMAs on
            # separate HWDGE rings so the halves stream out independently
            nc.vector.tensor_copy(stage[:, 0:512], apts[0][:])
            nc.sync.dma_start(out_d[qt * 128:(qt + 1) * 128, 0:512],
                              stage[:, 0:512])
            nc.scalar.activation(stage[:, 512:1024], apts[1][:],
                                 mybir.ActivationFunctionType.Copy)
            nc.scalar.dma_start(out_d[qt * 128:(qt + 1) * 128, 512:1024],
                                stage[:, 512:1024])

    # ---- per group: scores^T -> exp/mask/Z ; v-proj -> vp = v/Z ;
    #      then AV for the previous group's q-tiles (pipelined) ----
    for g in range(G):
        chunks = _score_chunks(g)
        nch = len(chunks)
        zp = zpool.tile([128, nch], F32, tag="zp", name="zp") if full else None
        for ci, (q0, w) in enumerate(chunks):
            pt = ps.tile([128, 512], F32, tag="ps")
            for dt_ in range(ET):
                lhs = (yt[dt_][:, g * 128:(g + 1) * 128] if full
                       else xk[dt_][:, g * 128:(g + 1) * 128])
                nc.tensor.matmul(
                    pt[:, :w],
                    lhs,
                    xt[dt_][:, q0:q0 + w],
                    start=(dt_ == 0), stop=(dt_ == ET - 1),
                )
            if full:
                if ci == 0:
                    # masked (diagonal) region = first 256 valid columns
                    nc.vector.tensor_add(pt[:, :256], pt[:, :256], mb_sb[:])
                nc.scalar.activation(
                    wt_sb[:, WOFF[g] + (q0 - 256 * g): WOFF[g] + (q0 - 256 * g) + w],
                    pt[:, :w],
                    Exp,
                    scale=float(SCALE),
                    accum_out=zp[:, ci:ci + 1],
                )

        # v projection for this group's 128 keys, accumulated in PSUM
        vpsum = vps.tile([128, D], F32, tag="vps")
        for e in range(ET):
            for oc in range(2):
                nc.tensor.matmul(
                    vpsum[:, oc * 512:(oc + 1) * 512],
                    xk[e][:, g * 128:(g + 1) * 128],
                    wvt[e][:, oc * 512:(oc + 1) * 512],
                    start=(e == 0), stop=(e == ET - 1),
                )
        if full:
            z = zpool.tile([128, 1], F32, tag="z")
            nc.vector.tensor_reduce(z[:], zp[:], axis=mybir.AxisListType.X,
                                    op=mybir.AluOpType.add)
            rz = zpool.tile([128, 1], F32, tag="rz")
            nc.vector.reciprocal(rz[:], z[:])
            vptile = vppool.tile([128, D], BF, tag="vp")
            # vp = v * (1/Z): PSUM fp32 -> SBUF bf16, per-partition scale
            nc.scalar.activation(vptile[:], vpsum[:], Copy, scale=rz[:])
        else:
            vptile = None
        vp[g] = vptile

        if g >= 1:
            emit_av(2 * (g - 1))
            emit_av(2 * (g - 1) + 1)

    emit_av(NT - 2)
    emit_av(NT - 1)


def build_program(with_loop=False, max_iters=64, stages="full"):
    """Build and compile the SPMD program. Returns the compiled Bacc."""
    nc = bacc.Bacc("TRN2", target_bir_lowering=False, debug=False, num_devices=8)
    xT_d = nc.dram_tensor("xT", [D, N], BF, kind="ExternalInput").ap()
    xkT_d = nc.dram_tensor("xkT", [D, D], BF, kind="ExternalInput").ap()
    a2_d = nc.dram_tensor("a2", [D, D], BF, kind="ExternalInput").ap()
    wvT_d = nc.dram_tensor("wvT", [D, D], BF, kind="ExternalInput").ap()
    mb_d = nc.dram_tensor("maskbias", [128, 256], F32, kind="ExternalInput").ap()
    out_d = nc.dram_tensor("out", [N, D], F32, kind="ExternalOutput").ap()
    if with_loop:
        n_d = nc.dram_tensor("niter", [1, 1], mybir.dt.int32,
                             kind="ExternalInput").ap()

    with tile.TileContext(nc) as tc:
        with ExitStack() as ctx:
            persist = ctx.enter_context(tc.tile_pool(name="persist", bufs=1))
            xkpool = ctx.enter_context(tc.tile_pool(name="xk", bufs=ET))
            xtpool = ctx.enter_context(tc.tile_pool(name="xT", bufs=ET))
            wpool = ctx.enter_context(tc.tile_pool(name="weights", bufs=ET))
            ypool = ctx.enter_context(tc.tile_pool(name="yT", bufs=ET))
            vppool = ctx.enter_context(tc.tile_pool(name="vp", bufs=G))
            zpool = ctx.enter_context(tc.tile_pool(name="z", bufs=3 * G))
            stpool = ctx.enter_context(tc.tile_pool(name="stage", bufs=4))
            ps = ctx.enter_context(tc.tile_pool(name="ps", bufs=6, space="PSUM"))
            vps = ctx.enter_context(tc.tile_pool(name="vps", bufs=1, space="PSUM"))

            mb_sb = persist.tile([128, 256], F32, tag="mb")
            # ACT's HWDGE ring: keeps the SP ring free for the xk/a2 pairs
            # that gate the first matmuls
            nc.scalar.dma_start(mb_sb[:], mb_d[:])
            wt_sb = persist.tile([128, WTOT], BF, tag="wt")

            pools = (xkpool, xtpool, wpool, ypool, vppool, zpool, stpool, ps, vps)
            aps = (xT_d, xkT_d, a2_d, wvT_d, mb_sb, wt_sb, out_d)

            if with_loop:
                n_sb = persist.tile([1, 1], mybir.dt.int32, tag="niter")
                nc.sync.dma_start(n_sb[:], n_d[:])
                regs = []
                with tc.tile_critical():
                    for e, eng in nc.engines.items():
                        r = eng.alloc_register(f"niter_{e.name}")
                        eng.reg_load(r, n_sb[0:1, 0:1])
                        regs.append(r)
                n_val = nc.snap(bass.RegisterHandles(regs), min_val=0,
                                max_val=max_iters)
                with tc.For_i(0, n_val, 1):
                    _emit_body(nc, tc, ctx, pools, aps, stages)
            else:
                _emit_body(nc, tc, ctx, pools, aps, stages)

    nc.compile()
    return nc


def prepare_in_maps(x, Wq, Wk, Wv, niter=None):
    """Host-side sharding: per-core input maps (8 cores)."""
    x = np.asarray(x, dtype=np.float32)
    A2 = (np.asarray(Wk, np.float32).T @ np.asarray(Wq, np.float32))  # [e, d]
    a2_bf = A2.astype(ml_dtypes.bfloat16)
    wvT_bf = np.asarray(Wv, np.float32).T.astype(ml_dtypes.bfloat16)  # [e, o]
    mb = []
    for h in range(2):
        i = np.arange(128)[:, None]
        j = np.arange(256)[None, :]
        mb.append(np.where(j >= 2 * i + h, 0.0, NEGBIG).astype(np.float32))
    in_maps = []
    for c in range(8):
        b, h = c // 2, c % 2
        xTb = x[b].T.astype(ml_dtypes.bfloat16)
        m = {
            "xT": xTb,
            "xkT": np.ascontiguousarray(xTb[:, h::2]),
            "a2": a2_bf,
            "wvT": wvT_bf,
            "maskbias": mb[h],
        }
        if niter is not None:
            m["niter"] = np.array([[niter]], dtype=np.int32)
        in_maps.append(m)
    return in_maps


_CACHE = {}


def kernel(x, Wq, Wk, Wv):
    if "nc" not in _CACHE:
        _CACHE["nc"] = build_program(with_loop=False)
    nc = _CACHE["nc"]
    in_maps = prepare_in_maps(x, Wq, Wk, Wv)
    res = run_bass_kernel_spmd(nc, in_maps, list(range(8)), trace=False)
    out = np.empty((B, N, D), np.float32)
    for b in range(B):
        out[b] = res.results[2 * b]["out"] + res.results[2 * b + 1]["out"]
    return out


# revision 10
# speedup vs baseline: 1.0247x; 1.0247x over previous
"""Causal self-attention (softmax over the QUERY axis) for Trainium2, 8 cores.

Reference semantics (note the quirk -- softmax over dim=1, the query axis):
    q = x @ Wq.T ; k = x @ Wk.T ; v = x @ Wv.T          (per batch)
    s[q_, k_] = <q[q_], k[k_]>,  masked -inf where k_ > q_
    attn = softmax(s / sqrt(D), axis=q_)                 (normalize per key column)
    out[q_, :] = sum_k attn[q_, k_] * v[k_, :]

Because the softmax normalizes each key COLUMN over queries, the whole thing
factors as  out = W @ (v / Z)  with
    W[k_, q_] = exp(s^T * scale) * causal_mask,   Z[k_] = sum_q W[k_, q_].

Algebraic folding: s[q_, k_] = x[q_] . A . x[k_]  with A = Wq^T @ Wk, so with
y = x_k @ A^T the scores come straight from x (no q/k projections needed):
    s^T[k_, q_] = sum_d y[k_, d] * x[q_, d].
A is computed once on the host.

Sharding: 8 cores = 4 batches x 2 key-shards.  Key columns are interleaved by
parity (core h in {0,1} owns original key positions 2*m + h) so the causal
work balances AND every core runs the identical program (pure SPMD); only the
input data differs per core.  Each core computes a partial output (sum over
its own keys); the host adds the two partials per batch.

Device layout (per core, b = batch, h = parity):
    xT   [D, N]  bf16  x[b].T
    a2   [D, D]  bf16  A^T = Wk^T @ Wq   (layout [e, d])
    wvT  [D, D]  bf16  Wv.T              (layout [e, o])
    maskbias [128, 256] f32  0 where valid, -1e9 where masked (depends on h)
    out  [N, D]  f32   partial output

Pipeline structure (v3):
  - input DMAs interleaved (xk[e]/a2[e] pairs first) so the y-projection's
    accumulation chains start as soon as the first pair lands;
  - PE warm-up matmuls on a zeroed tile during the initial DMA window
    (HAM clock-gate releases before the real matmuls start);
  - v-projection inside the per-group loop, accumulated in a 2-bank PSUM
    tile and scaled by 1/Z directly PSUM->SBUF(bf16) on the Scalar engine
    (activation Copy with per-partition scale) -- no fp32 v copy;
  - AV for q-tiles of group g-1 emitted at group g (software pipelining);
  - matmul free-dim chunk width CW is tunable: HW measures better
    per-column throughput at narrower chunks under the machine's power
    throttle (~0.62 ns/col at N=128 vs ~0.72 at N=512).

All matmul inputs are bf16 (PE full rate), accumulation fp32 in PSUM.
"""
import numpy as np
import ml_dtypes
from contextlib import ExitStack

import concourse.bass as bass
import concourse.tile as tile
import concourse.bacc as bacc
import concourse.mybir as mybir
from concourse.bass_utils import run_bass_kernel_spmd

B, N, D = 4, 2048, 1024
NT = N // 128          # 16 query tiles
ET = D // 128          # 8 contraction tiles
G = 8                  # key groups per core (128 interleaved keys each)
SCALE = 1.0 / np.sqrt(D).astype(np.float32)
NEGBIG = -1.0e9
CW_DEFAULT = 512       # matmul free-dim chunk width

BF = mybir.dt.bfloat16
F32 = mybir.dt.float32

# packed offsets of each group's score row-block inside the wT buffer
WOFF = []
_o = 0
for _g in range(G):
    WOFF.append(_o)
    _o += N - 256 * _g
WTOT = _o  # 9216


def _score_chunks(g, cw):
    """(q0, width) chunks covering the valid span [256g, N) of group g.
    The first chunk always contains the 256 masked boundary columns."""
    width = N - 256 * g
    q0 = 256 * g
    chunks = []
    rem = width % cw
    if rem:
        chunks.append((q0, rem))
        q0 += rem
    while q0 < N:
        chunks.append((q0, cw))
        q0 += cw
    return chunks


def _emit_body(nc, tc, ctx, pools, aps, stages="full", cw=CW_DEFAULT):
    (xkpool, xtpool, wpool, ypool, vppool, zpool, stpool, ps, vps) = pools
    (xT_d, xkT_d, a2_d, wvT_d, mb_sb, wt_sb, out_d) = aps
    full = stages == "full"
    Exp = mybir.ActivationFunctionType.Exp
    Copy = mybir.ActivationFunctionType.Copy
    NCH = D // cw            # chunks per 1024-wide span

    # ---- PE warm-up: dummy matmuls on a zeroed tile keep the PE HAM busy
    #      while the first input DMAs land ----
    warm = zpool.tile([128, cw], BF, tag="warm", name="warm")
    nc.gpsimd.memset(warm[:], 0)
    wps = ps.tile([128, cw], F32, tag="ps", name="wps")
    NWARM = 8 * 512 // cw
    for i in range(NWARM):
        nc.tensor.matmul(wps[:], warm[:, :128], warm[:],
                         start=(i == 0), stop=(i == NWARM - 1))

    # ---- loads: xk/a2 pairs (y-proj can start after the first pair),
    #      then xT (needed by scores at ~27us), then wvT (v-proj) ----
    xk, a2t = [], []
    for e in range(ET):
        t1 = xkpool.tile([128, D], BF, tag="xk")
        nc.sync.dma_start(t1[:], xkT_d[e * 128:(e + 1) * 128, :])
        xk.append(t1)
        t2 = wpool.tile([128, D], BF, tag="a2")
        nc.sync.dma_start(t2[:], a2_d[e * 128:(e + 1) * 128, :])
        a2t.append(t2)
    xt = []
    for t in range(ET):
        xtile = xtpool.tile([128, N], BF, tag="xt")
        nc.sync.dma_start(xtile[:], xT_d[t * 128:(t + 1) * 128, :])
        xt.append(xtile)
    wvt = []
    for e in range(ET):
        w = wpool.tile([128, D], BF, tag="wv")
        nc.sync.dma_start(w[:], wvT_d[e * 128:(e + 1) * 128, :])
        wvt.append(w)

    # ---- y projection: yT[dt_][:, m] = sum_e a2[e, d_blk] * xkT[e, m] ----
    yt = []
    for dt_ in range(ET):
        ytile = ypool.tile([128, D], BF, tag="yt")
        pts = [ps.tile([128, cw], F32, tag="ps", name=f"pt{c}")
               for c in range(NCH)]
        for e in range(ET):
            for c in range(NCH):
                nc.tensor.matmul(
                    pts[c][:],
                    a2t[e][:, dt_ * 128:(dt_ + 1) * 128],
                    xk[e][:, c * cw:(c + 1) * cw],
                    start=(e == 0), stop=(e == ET - 1),
                )
        if full:
            for c in range(NCH):
                nc.vector.tensor_copy(ytile[:, c * cw:(c + 1) * cw], pts[c][:])
        yt.append(ytile)

    vp = [None] * G

    def emit_av(qt):
        """AV for q-tile qt: out[128q, D] = sum_gg wt_gg^T @ vp_gg."""
        g_qt = qt // 2
        stage = stpool.tile([128, D], F32, tag="st")
        apts = [ps.tile([128, cw], F32, tag="ps", name=f"apt{c}")
                for c in range(NCH)]
        # For the very last q-tile, run the oc-chains back-to-back (oc-outer)
        # so earlier chunks' copy+DMA overlap the later chunks' matmuls.
        if qt == NT - 1:
            order = [(gg, oc) for oc in range(NCH) for gg in range(g_qt + 1)]
        else:
            order = [(gg, oc) for gg in range(g_qt + 1) for oc in range(NCH)]
        for gg, oc in order:
            if full:
                lhs = wt_sb[:, WOFF[gg] + 128 * qt - 256 * gg:
                               WOFF[gg] + 128 * qt - 256 * gg + 128]
                rhs = vp[gg][:, oc * cw:(oc + 1) * cw]
            else:
                lhs = xk[gg][:, :128]
                rhs = xk[gg][:, oc * cw:(oc + 1) * cw]
            nc.tensor.matmul(apts[oc][:], lhs, rhs,
                             start=(gg == 0), stop=(gg == g_qt))
        if full or qt == NT - 1:
            # chunk copies alternate DVE/ACT; two out-DMAs on separate
            # HWDGE rings so the halves stream out independently
            for oc in range(NCH):
                dst = stage[:, oc * cw:(oc + 1) * cw]
                if oc % 2 == 0:
                    nc.vector.tensor_copy(dst, apts[oc][:])
                else:
                    nc.scalar.activation(dst, apts[oc][:], Copy)
            nc.sync.dma_start(out_d[qt * 128:(qt + 1) * 128, 0:512],
                              stage[:, 0:512])
            nc.scalar.dma_start(out_d[qt * 128:(qt + 1) * 128, 512:1024],
                                stage[:, 512:1024])

    # ---- per group: scores^T -> exp/mask/Z ; v-proj -> vp = v/Z ;
    #      then AV for the previous group's q-tiles (pipelined) ----
    for g in range(G):
        chunks = _score_chunks(g, cw)
        nch = len(chunks)
        zp = zpool.tile([128, nch], F32, tag="zp", name="zp") if full else None
        for ci, (q0, w) in enumerate(chunks):
            pt = ps.tile([128, cw], F32, tag="ps")
            for dt_ in range(ET):
                lhs = (yt[dt_][:, g * 128:(g + 1) * 128] if full
                       else xk[dt_][:, g * 128:(g + 1) * 128])
                nc.tensor.matmul(
                    pt[:, :w],
                    lhs,
                    xt[dt_][:, q0:q0 + w],
                    start=(dt_ == 0), stop=(dt_ == ET - 1),
                )
            if full:
                if ci == 0:
                    # masked (diagonal) region = first 256 valid columns
                    nc.vector.tensor_add(pt[:, :256], pt[:, :256], mb_sb[:])
                nc.scalar.activation(
                    wt_sb[:, WOFF[g] + (q0 - 256 * g): WOFF[g] + (q0 - 256 * g) + w],
                    pt[:, :w],
                    Exp,
                    scale=float(SCALE),
                    accum_out=zp[:, ci:ci + 1],
                )

        # v projection for this group's 128 keys, accumulated in PSUM
        vpsum = vps.tile([128, D], F32, tag="vps")
        for e in range(ET):
            for oc in range(NCH):
                nc.tensor.matmul(
                    vpsum[:, oc * cw:(oc + 1) * cw],
                    xk[e][:, g * 128:(g + 1) * 128],
                    wvt[e][:, oc * cw:(oc + 1) * cw],
                    start=(e == 0), stop=(e == ET - 1),
                )
        if full:
            z = zpool.tile([128, 1], F32, tag="z")
            nc.vector.tensor_reduce(z[:], zp[:], axis=mybir.AxisListType.X,
                                    op=mybir.AluOpType.add)
            rz = zpool.tile([128, 1], F32, tag="rz")
            nc.vector.reciprocal(rz[:], z[:])
            vptile = vppool.tile([128, D], BF, tag="vp")
            # vp = v * (1/Z): PSUM fp32 -> SBUF bf16, per-partition scale
            nc.scalar.activation(vptile[:], vpsum[:], Copy, scale=rz[:])
        else:
            vptile = None
        vp[g] = vptile

        if g >= 1:
            emit_av(2 * (g - 1))
            emit_av(2 * (g - 1) + 1)

    emit_av(NT - 2)
    emit_av(NT - 1)


def build_program(with_loop=False, max_iters=64, stages="full", cw=CW_DEFAULT):
    """Build and compile the SPMD program. Returns the compiled Bacc."""
    nc = bacc.Bacc("TRN2", target_bir_lowering=False, debug=False, num_devices=8)
    xT_d = nc.dram_tensor("xT", [D, N], BF, kind="ExternalInput").ap()
    xkT_d = nc.dram_tensor("xkT", [D, D], BF, kind="ExternalInput").ap()
    a2_d = nc.dram_tensor("a2", [D, D], BF, kind="ExternalInput").ap()
    wvT_d = nc.dram_tensor("wvT", [D, D], BF, kind="ExternalInput").ap()
    mb_d = nc.dram_tensor("maskbias", [128, 256], F32, kind="ExternalInput").ap()
    out_d = nc.dram_tensor("out", [N, D], F32, kind="ExternalOutput").ap()
    if with_loop:
        n_d = nc.dram_tensor("niter", [1, 1], mybir.dt.int32,
                             kind="ExternalInput").ap()

    with tile.TileContext(nc) as tc:
        with ExitStack() as ctx:
            persist = ctx.enter_context(tc.tile_pool(name="persist", bufs=1))
            xkpool = ctx.enter_context(tc.tile_pool(name="xk", bufs=ET))
            xtpool = ctx.enter_context(tc.tile_pool(name="xT", bufs=ET))
            wpool = ctx.enter_context(tc.tile_pool(name="weights", bufs=ET))
            ypool = ctx.enter_context(tc.tile_pool(name="yT", bufs=ET))
            vppool = ctx.enter_context(tc.tile_pool(name="vp", bufs=G))
            zpool = ctx.enter_context(tc.tile_pool(name="z", bufs=3 * G))
            stpool = ctx.enter_context(tc.tile_pool(name="stage", bufs=4))
            ps = ctx.enter_context(tc.tile_pool(name="ps", bufs=6, space="PSUM"))
            vps = ctx.enter_context(tc.tile_pool(name="vps", bufs=1, space="PSUM"))

            mb_sb = persist.tile([128, 256], F32, tag="mb")
            # ACT's HWDGE ring: keeps the SP ring free for the xk/a2 pairs
            # that gate the first matmuls
            nc.scalar.dma_start(mb_sb[:], mb_d[:])
            wt_sb = persist.tile([128, WTOT], BF, tag="wt")

            pools = (xkpool, xtpool, wpool, ypool, vppool, zpool, stpool, ps, vps)
            aps = (xT_d, xkT_d, a2_d, wvT_d, mb_sb, wt_sb, out_d)

            if with_loop:
                n_sb = persist.tile([1, 1], mybir.dt.int32, tag="niter")
                nc.sync.dma_start(n_sb[:], n_d[:])
                regs = []
                with tc.tile_critical():
                    for e, eng in nc.engines.items():
                        r = eng.alloc_register(f"niter_{e.name}")
                        eng.reg_load(r, n_sb[0:1, 0:1])
                        regs.append(r)
                n_val = nc.snap(bass.RegisterHandles(regs), min_val=0,
                                max_val=max_iters)
                with tc.For_i(0, n_val, 1):
                    _emit_body(nc, tc, ctx, pools, aps, stages, cw)
            else:
                _emit_body(nc, tc, ctx, pools, aps, stages, cw)

    nc.compile()
    return nc


def prepare_in_maps(x, Wq, Wk, Wv, niter=None):
    """Host-side sharding: per-core input maps (8 cores)."""
    x = np.asarray(x, dtype=np.float32)
    A2 = (np.asarray(Wk, np.float32).T @ np.asarray(Wq, np.float32))  # [e, d]
    a2_bf = A2.astype(ml_dtypes.bfloat16)
    wvT_bf = np.asarray(Wv, np.float32).T.astype(ml_dtypes.bfloat16)  # [e, o]
    mb = []
    for h in range(2):
        i = np.arange(128)[:, None]
        j = np.arange(256)[None, :]
        mb.append(np.where(j >= 2 * i + h, 0.0, NEGBIG).astype(np.float32))
    in_maps = []
    for c in range(8):
        b, h = c // 2, c % 2
        xTb = x[b].T.astype(ml_dtypes.bfloat16)
        m = {
            "xT": xTb,
            "xkT": np.ascontiguousarray(xTb[:, h::2]),
            "a2": a2_bf,
            "wvT": wvT_bf,
            "maskbias": mb[h],
        }
        if niter is not None:
            m["niter"] = np.array([[niter]], dtype=np.int32)
        in_maps.append(m)
    return in_maps


_CACHE = {}


def kernel(x, Wq, Wk, Wv):
    if "nc" not in _CACHE:
        _CACHE["nc"] = build_program(with_loop=False)
    nc = _CACHE["nc"]
    in_maps = prepare_in_maps(x, Wq, Wk, Wv)
    res = run_bass_kernel_spmd(nc, in_maps, list(range(8)), trace=False)
    out = np.empty((B, N, D), np.float32)
    for b in range(B):
        out[b] = res.results[2 * b]["out"] + res.results[2 * b + 1]["out"]
    return out
